# revision 16
# baseline (speedup 1.0000x reference)
"""Trainium2 Bass kernel for nn_MeshGraphBlock (GNN message-passing block).

Computes, for x:[B,N,D], edges (src,dst):[E], degree:[N]:
    neighbor = scatter_add(x[:, src, :] -> dst) / clip(degree, 1)
    h  = concat(LN(x; sn_g, sn_b), LN(neighbor; nn_g, nn_b))   # [B,N,2D]
    h  = gelu_erf(h @ W1 + b1)                                  # [B,N,2D]
    y  = x + h @ W2 + b2                                        # [B,N,D]

LayerNorm is scale-invariant and `neighbor` only feeds the LN, so the
1/deg factor cancels exactly (up to an eps-term ~1e-4 rel) and is dropped.

Strategy (8 NeuronCores, SPMD):
 - Destination-node tiles (128 nodes each) are assigned to cores via
   sorted round-robin so every core sees the same per-position edge-tile
   counts (the single compiled program is uniform; only data differs).
 - Host pre-sorts edges by dst, packs x (both batches side by side) as a
   bf16 [N,2D] table, and emits per-core gather indices (int16, split in
   two tables to stay under the 32767 index limit).
 - Edge messages are gathered with few, large dma_gather calls (grouped
   across GROUP dst positions; SWDGE ring enlarged via
   dynamic_dma_scratch_size) and scatter-added into 128-dst PSUM
   accumulators via one-hot "selection matrix" matmuls on the PE.
 - All matmul operands are bf16 (1 PE cycle/row); residual x and the b2
   bias are folded into the mm2 PSUM chain as extra matmuls; rstd is
   computed on DVE with a fused (var+eps)^-0.5 tensor_scalar (no
   activation-table thrash).
"""

import math

import numpy as np
import ml_dtypes

P = 128
NCORES = 8
SPLIT = 32768           # int16 gather-index limit
MAX_TILES_PER_CALL = 24  # 3072 idxs per dma_gather (enlarged SWDGE ring)
TUNE = dict(gpool=3, spool=6, wp=6, hp=4, group=3, scratch=65536)

_CACHE = {}


def _prep(x, edge_src, edge_dst):
    """Host-side sharding. Returns (structure, per-core inputs, assembly map)."""
    Bb, N, D = x.shape
    es = np.asarray(edge_src).astype(np.int64).ravel()
    ed = np.asarray(edge_dst).astype(np.int64).ravel()

    ntiles = math.ceil(N / P)
    ntiles_pad = math.ceil(ntiles / NCORES) * NCORES
    NTC = ntiles_pad // NCORES
    GROUP = TUNE["group"]

    order = np.argsort(ed, kind="stable")
    ed_s = ed[order]
    es_s = es[order]
    bounds = np.searchsorted(ed_s, np.arange(ntiles_pad + 1) * P)

    counts = bounds[1:] - bounds[:-1]
    ranked = np.argsort(-counts, kind="stable")
    # tile ranked[i] -> core i % 8, position i // 8
    tids = [[0] * NTC for _ in range(NCORES)]
    for i, t in enumerate(ranked):
        tids[i % NCORES][i // NCORES] = int(t)

    # per (core, pos): split into G0 (src < SPLIT) and G1
    g0i, g1i, dli = {}, {}, {}
    for c in range(NCORES):
        for k in range(NTC):
            t = tids[c][k]
            a, b = bounds[t], bounds[t + 1]
            srcs = es_s[a:b]
            dloc = (ed_s[a:b] - t * P).astype(np.int64)
            m0 = srcs < SPLIT
            g0i[c, k] = srcs[m0].astype(np.int64)
            g1i[c, k] = (srcs[~m0] - SPLIT).astype(np.int64)
            dli[c, k] = (dloc[m0], dloc[~m0])

    T0 = [max(math.ceil(len(g0i[c, k]) / P) for c in range(NCORES)) for k in range(NTC)]
    T1 = [max(math.ceil(len(g1i[c, k]) / P) for c in range(NCORES)) for k in range(NTC)]

    # Group-level layout: for each group of GROUP positions, the tile
    # stream is [k0 A-tiles, k1 A-tiles, ..., k0 B-tiles, k1 B-tiles, ...]
    # so one (or few) gather calls per source table cover the whole group.
    TTOT = sum(T0) + sum(T1)
    idx_flat = np.zeros((NCORES, TTOT * P), dtype=np.int16)
    dl_flat = np.full((NCORES, TTOT * P), -1.0, dtype=np.float32)
    groups = []   # per group: dict(calls=[(src, slot_off, ntiles, idx_off)],
                  #                pos=[(k, a_off, b_off)], tg=total tiles)
    tile_off = 0  # global tile counter (indexes dlb columns / idx stream)
    for k0 in range(0, NTC, GROUP):
        gs = min(GROUP, NTC - k0)
        ginfo = dict(calls=[], pos=[], base=tile_off)
        # A tiles then B tiles, each position-ordered
        offs = {}
        so = 0
        for grp, Tarr in ((0, T0), (1, T1)):
            grp_start_tile = tile_off
            grp_start_slot = so
            ntile_grp = 0
            for gi in range(gs):
                k = k0 + gi
                T = Tarr[k]
                offs[k, grp] = so
                for c in range(NCORES):
                    ii = g0i[c, k] if grp == 0 else g1i[c, k]
                    dd = dli[c, k][grp]
                    o = tile_off * P
                    idx_flat[c, o : o + len(ii)] = ii.astype(np.int16)
                    dl_flat[c, o : o + len(dd)] = dd.astype(np.float32)
                tile_off += T
                so += T
                ntile_grp += T
            # split the group's tiles for this source into ring-sized calls
            done = 0
            while done < ntile_grp:
                nt = min(ntile_grp - done, MAX_TILES_PER_CALL)
                ginfo["calls"].append(
                    ("A" if grp == 0 else "B", grp_start_slot + done,
                     nt, (grp_start_tile + done) * P)
                )
                done += nt
        for gi in range(gs):
            k = k0 + gi
            ginfo["pos"].append((k, offs[k, 0], offs[k, 1]))
        ginfo["tg"] = so
        groups.append(ginfo)
    assert tile_off == TTOT

    # wrapped int16 idx layout: [128, TTOT*P/16]
    idx_wrapped = np.stack(
        [np.tile(idx_flat[c].reshape(-1, 16).T, (8, 1)) for c in range(NCORES)]
    )
    dlb = np.stack(
        [np.ascontiguousarray(dl_flat[c].reshape(TTOT, P).T)
         for c in range(NCORES)]
    )  # [NCORES, 128, TTOT] float32

    # per-core x slices ([NTC*128, 2D]) bf16
    xs = np.zeros((NCORES, NTC * P, 2 * D), dtype=ml_dtypes.bfloat16)
    xf = np.asarray(x, dtype=np.float32)
    for c in range(NCORES):
        for k in range(NTC):
            t = tids[c][k]
            n0 = t * P
            n1 = min(n0 + P, N)
            if n1 <= n0:
                continue
            xs[c, k * P : k * P + (n1 - n0), :D] = xf[0, n0:n1, :]
            xs[c, k * P : k * P + (n1 - n0), D:] = xf[1, n0:n1, :]

    # packed gather tables (both batches side by side), bf16
    xpack = np.concatenate([xf[0], xf[1]], axis=1).astype(ml_dtypes.bfloat16)
    xpa = np.ascontiguousarray(xpack[:SPLIT])
    xpb = np.ascontiguousarray(xpack[SPLIT:]) if N > SPLIT else None

    slots_max = max(t0 + t1 for t0, t1 in zip(T0, T1))
    tg_max = max(g["tg"] for g in groups)
    struct = dict(NTC=NTC, T0=tuple(T0), T1=tuple(T1), TTOT=TTOT,
                  groups=groups, slots_max=slots_max, tg_max=tg_max,
                  NA=xpa.shape[0], NB=(xpb.shape[0] if xpb is not None else 0),
                  D=D, Bb=Bb)
    percore = dict(idx=idx_wrapped, dlb=dlb, xs=xs)
    shared = dict(xpa=xpa, xpb=xpb)
    return struct, percore, shared, tids, N


def _build(struct):
    import concourse.bacc as bacc
    import concourse.tile as tile
    from concourse import bass, mybir
    from concourse.masks import make_identity

    NTC, T0, T1, TTOT = struct["NTC"], struct["T0"], struct["T1"], struct["TTOT"]
    groups = struct["groups"]
    D = struct["D"]
    D2 = 2 * D
    TOTCOLS = TTOT * P // 16
    slots_max = struct["slots_max"]
    tg_max = struct["tg_max"]
    f32, bf16, i16 = mybir.dt.float32, mybir.dt.bfloat16, mybir.dt.int16

    nc = bacc.Bacc("TRN2", target_bir_lowering=False, debug=False,
                   dynamic_dma_scratch_size=TUNE["scratch"])
    d_xpa = nc.dram_tensor("xpa", [struct["NA"], D2], bf16, kind="ExternalInput")
    d_xpb = (nc.dram_tensor("xpb", [struct["NB"], D2], bf16, kind="ExternalInput")
             if struct["NB"] else None)
    d_xs = nc.dram_tensor("xs", [NTC * P, D2], bf16, kind="ExternalInput")
    d_idx = nc.dram_tensor("idx", [P, TOTCOLS], i16, kind="ExternalInput")
    d_dlb = nc.dram_tensor("dlb", [P, TTOT], f32, kind="ExternalInput")
    d_w1 = nc.dram_tensor("w1", [D2, D2], f32, kind="ExternalInput")
    d_w2 = nc.dram_tensor("w2", [D2, D], bf16, kind="ExternalInput")
    d_b1 = nc.dram_tensor("b1r", [P, 2], f32, kind="ExternalInput")
    d_b2 = nc.dram_tensor("b2r", [1, D2], bf16, kind="ExternalInput")
    d_gx = nc.dram_tensor("gx", [P, 1], f32, kind="ExternalInput")
    d_gn = nc.dram_tensor("gn", [P, 1], f32, kind="ExternalInput")
    d_bx = nc.dram_tensor("bx", [P, 1], f32, kind="ExternalInput")
    d_bn = nc.dram_tensor("bn", [P, 1], f32, kind="ExternalInput")
    d_y = nc.dram_tensor("y", [NTC * P, D2], bf16, kind="ExternalOutput")

    with tile.TileContext(nc) as tc:
        with (
            tc.tile_pool(name="const", bufs=1) as cp,
            tc.tile_pool(name="gath", bufs=TUNE["gpool"]) as gpool,
            tc.tile_pool(name="sel", bufs=TUNE["spool"]) as spool,
            tc.tile_pool(name="work", bufs=TUNE["wp"]) as wp,
            tc.tile_pool(name="ht", bufs=TUNE["hp"]) as hp,
            tc.tile_pool(name="nbps", bufs=2, space="PSUM") as nbps,
            tc.tile_pool(name="trps", bufs=2, space="PSUM") as trps,
            tc.tile_pool(name="mm1ps", bufs=2, space="PSUM") as mm1ps,
            tc.tile_pool(name="mm2ps", bufs=2, space="PSUM") as mm2ps,
        ):
            # ---- one-time constants ----
            idx_sb = cp.tile([P, TOTCOLS], i16)
            nc.sync.dma_start(idx_sb[:], d_idx.ap())
            dlb_sb = cp.tile([P, TTOT], f32)
            nc.sync.dma_start(dlb_sb[:], d_dlb.ap())

            ident = cp.tile([P, P], bf16)
            make_identity(nc, ident[:])
            iota1 = cp.tile([P, P], bf16)
            nc.gpsimd.iota(iota1[:], pattern=[[1, P]], base=0,
                           channel_multiplier=0,
                           allow_small_or_imprecise_dtypes=True)

            gx_sb = cp.tile([P, 1], f32); nc.sync.dma_start(gx_sb[:], d_gx.ap())
            gn_sb = cp.tile([P, 1], f32); nc.sync.dma_start(gn_sb[:], d_gn.ap())
            bx_sb = cp.tile([P, 1], f32); nc.sync.dma_start(bx_sb[:], d_bx.ap())
            bn_sb = cp.tile([P, 1], f32); nc.sync.dma_start(bn_sb[:], d_bn.ap())
            b1r_sb = cp.tile([P, 2], f32); nc.sync.dma_start(b1r_sb[:], d_b1.ap())
            b2r_sb = cp.tile([1, D2], bf16); nc.sync.dma_start(b2r_sb[:], d_b2.ap())
            ones1 = cp.tile([1, P], bf16)
            nc.vector.memset(ones1[:], 1.0)

            # W1 tiles [k-tile][j-tile], gamma-scaled bf16 copies, W2 bf16
            w1t = [[cp.tile([P, P], f32, name=f"w1t{kt}{jt}") for jt in range(2)]
                   for kt in range(2)]
            w1s = [[cp.tile([P, P], bf16, name=f"w1s{kt}{jt}") for jt in range(2)]
                   for kt in range(2)]
            gam = [gx_sb, gn_sb]
            for kt in range(2):
                for jt in range(2):
                    nc.sync.dma_start(
                        w1t[kt][jt][:],
                        d_w1[kt * P : (kt + 1) * P, jt * P : (jt + 1) * P],
                    )
                    nc.vector.tensor_scalar_mul(
                        w1s[kt][jt][:], w1t[kt][jt][:], gam[kt][:]
                    )
            w2t = [cp.tile([P, P], bf16, name=f"w2t{kt}") for kt in range(2)]
            for kt in range(2):
                nc.sync.dma_start(w2t[kt][:], d_w2[kt * P : (kt + 1) * P, :])

            # b1_eff = b1 + beta_cat @ W1  (per-partition layout [128, j-tile])
            bet = [bx_sb, bn_sb]
            b1b_ps = mm1ps.tile([P, 2], f32, space="PSUM", tag="m1")
            for jt in range(2):
                for kt in range(2):
                    nc.tensor.matmul(
                        b1b_ps[:, jt : jt + 1], lhsT=w1t[kt][jt][:],
                        rhs=bet[kt][:], start=(kt == 0), stop=(kt == 1),
                    )
            b1e_sb = cp.tile([P, 2], f32)
            nc.vector.tensor_add(b1e_sb[:], b1b_ps[:], b1r_sb[:])

            i32 = mybir.dt.int32
            GROUP = max(len(g["pos"]) for g in groups)
            magic = cp.tile([P, 4 * GROUP], i32)
            nc.vector.memset(magic[:], 0x5F3759DF)

            # ---- main loop: software-pipelined emission ----
            # Phase A of group i is emitted BEFORE phase B of group i-1 so
            # every engine queue has the next group's work ahead of any
            # instruction stalled on the current group's rstd barrier.
            def phase_a(ginfo):
                base = ginfo["base"]
                gs = len(ginfo["pos"])
                nb_t, xs_t = {}, {}
                mvg = wp.tile([P, 4 * gs, 2], f32, tag="mvg", bufs=2,
                              name=f"mvg{base}")

                # group gather: few large calls
                g = gpool.tile([P, tg_max, D2], bf16, tag="g", name=f"g{base}")
                for (srcg, so, nt, io) in ginfo["calls"]:
                    src_t = d_xpa if srcg == "A" else d_xpb
                    nc.gpsimd.dma_gather(
                        g[:, so : so + nt, :], src_t.ap(),
                        idx_sb[:, io // 16 : (io + nt * P) // 16],
                        nt * P, nt * P, D2, single_packet=False,
                    )

                # phase A: aggregate neighbors + stats
                for gi, (k, a_off, b_off) in enumerate(ginfo["pos"]):
                    slots = T0[k] + T1[k]
                    nb_sb = wp.tile([P, D2], bf16, tag="nb", bufs=2 * TUNE["group"] + 1,
                                    name=f"nb{k}")
                    nb_t[k] = nb_sb
                    if slots == 0:
                        nc.vector.memset(nb_sb[:], 0.0)
                    else:
                        # (g-slot index, dlb column) per slot of this position
                        slist = (
                            [(a_off + t, base + a_off + t) for t in range(T0[k])]
                            + [(b_off + t, base + b_off + t) for t in range(T1[k])]
                        )
                        S = spool.tile([P, slots_max, P], bf16, tag="S",
                                       name=f"S{k}")
                        for i, (gslot, dcol) in enumerate(slist):
                            nc.vector.tensor_scalar(
                                out=S[:, i, :],
                                in0=iota1[:],
                                scalar1=dlb_sb[:, dcol : dcol + 1],
                                scalar2=None,
                                op0=mybir.AluOpType.is_equal,
                            )
                        nb_ps = nbps.tile([P, D2], f32, space="PSUM", tag="nbp",
                                          name=f"nbp{k}")
                        for i, (gslot, dcol) in enumerate(slist):
                            nc.tensor.matmul(
                                nb_ps[:], lhsT=S[:, i, :], rhs=g[:, gslot, :],
                                start=(i == 0), stop=(i == len(slist) - 1),
                            )
                        nc.scalar.copy(nb_sb[:], nb_ps[:])

                    xs_sb = wp.tile([P, D2], bf16, tag="xs", bufs=2 * TUNE["group"] + 1,
                                    name=f"xs{k}")
                    xs_t[k] = xs_sb
                    nc.sync.dma_start(xs_sb[:], d_xs[k * P : (k + 1) * P, :])
                    stx = wp.tile([P, 2, 6], f32, tag="stx", name=f"stx{k}")
                    stn = wp.tile([P, 2, 6], f32, tag="stn", name=f"stn{k}")
                    for b in range(2):
                        nc.vector.bn_stats(stx[:, b, :], xs_sb[:, b * D : (b + 1) * D])
                        nc.vector.bn_stats(stn[:, b, :], nb_sb[:, b * D : (b + 1) * D])
                    nc.vector.bn_aggr(mvg[:, 4 * gi + 0, :], stx[:, 0:1, :])
                    nc.vector.bn_aggr(mvg[:, 4 * gi + 1, :], stx[:, 1:2, :])
                    nc.vector.bn_aggr(mvg[:, 4 * gi + 2, :], stn[:, 0:1, :])
                    nc.vector.bn_aggr(mvg[:, 4 * gi + 3, :], stn[:, 1:2, :])

                # group-level rstd = (var + eps)^-0.5 on DVE via the
                # bit-trick rsqrt + 2 Newton iterations (no act-table load)
                rsg = wp.tile([P, 4 * gs], f32, tag="rsg", bufs=2, name=f"rsg{base}")
                veps = wp.tile([P, 4 * gs], f32, tag="veps", bufs=2,
                               name=f"veps{base}")
                ytmp = wp.tile([P, 4 * gs], f32, tag="ytmp", bufs=2,
                               name=f"ytmp{base}")
                ttmp = wp.tile([P, 4 * gs], f32, tag="ttmp", bufs=2,
                               name=f"ttmp{base}")
                nc.vector.tensor_scalar_add(veps[:], mvg[:, :, 1], 1e-5)
                # y0 = bitcast(magic - (bitcast(veps) >> 1))
                nc.vector.tensor_scalar(
                    out=ytmp[:].bitcast(i32), in0=veps[:].bitcast(i32),
                    scalar1=1, scalar2=None,
                    op0=mybir.AluOpType.arith_shift_right,
                )
                nc.vector.tensor_tensor(
                    out=ytmp[:].bitcast(i32), in0=magic[:, : 4 * gs],
                    in1=ytmp[:].bitcast(i32), op=mybir.AluOpType.subtract,
                )
                for _ in range(1):  # y <- y * (1.5 - 0.5 * veps * y^2)
                    nc.vector.tensor_tensor(
                        out=ttmp[:], in0=ytmp[:], in1=ytmp[:],
                        op=mybir.AluOpType.mult,
                    )
                    nc.vector.tensor_tensor(
                        out=ttmp[:], in0=ttmp[:], in1=veps[:],
                        op=mybir.AluOpType.mult,
                    )
                    nc.vector.tensor_scalar(
                        out=ttmp[:], in0=ttmp[:], scalar1=-0.5, scalar2=1.5,
                        op0=mybir.AluOpType.mult, op1=mybir.AluOpType.add,
                    )
                    nc.vector.tensor_tensor(
                        out=ytmp[:], in0=ytmp[:], in1=ttmp[:],
                        op=mybir.AluOpType.mult,
                    )
                nc.vector.tensor_copy(rsg[:], ytmp[:])
                return dict(nb_t=nb_t, xs_t=xs_t, mvg=mvg, rsg=rsg)

            # phase B: normalize + MLP + residual
            def phase_b(ginfo, st):
                nb_t, xs_t = st["nb_t"], st["xs_t"]
                mvg, rsg = st["mvg"], st["rsg"]
                for gi, (k, a_off, b_off) in enumerate(ginfo["pos"]):
                    nb_sb, xs_sb = nb_t[k], xs_t[k]
                    hx = wp.tile([P, D2], bf16, tag="hx", name=f"hx{k}")
                    hn = wp.tile([P, D2], bf16, tag="hn", name=f"hn{k}")
                    for b in range(2):
                        nc.vector.tensor_scalar(
                            out=hx[:, b * D : (b + 1) * D],
                            in0=xs_sb[:, b * D : (b + 1) * D],
                            scalar1=mvg[:, 4 * gi + b, 0:1],
                            scalar2=rsg[:, 4 * gi + b : 4 * gi + b + 1],
                            op0=mybir.AluOpType.subtract,
                            op1=mybir.AluOpType.mult,
                        )
                        nc.vector.tensor_scalar(
                            out=hn[:, b * D : (b + 1) * D],
                            in0=nb_sb[:, b * D : (b + 1) * D],
                            scalar1=mvg[:, 4 * gi + 2 + b, 0:1],
                            scalar2=rsg[:, 4 * gi + 2 + b : 4 * gi + 3 + b],
                            op0=mybir.AluOpType.subtract,
                            op1=mybir.AluOpType.mult,
                        )

                    # feature-major h via PE transposes into ONE [128,512]
                    # PSUM tile (1 bank -> true depth-2 pipelining); halves
                    # kt=0 -> x, kt=1 -> nb, both batches side by side
                    hTc = hp.tile([P, 2, D2], bf16, name=f"hTc_{k}", tag="hTc")
                    tp = trps.tile([P, 2, D2], bf16, space="PSUM", tag="tr",
                                   name=f"tr_{k}")
                    for kt, srct in ((0, hx), (1, hn)):
                        for b in range(2):
                            nc.tensor.transpose(
                                tp[:, kt, b * D : (b + 1) * D],
                                srct[:, b * D : (b + 1) * D], ident[:],
                            )
                    nc.scalar.copy(hTc[:], tp[:])

                    gsb = hp.tile([P, 2, D2], bf16, name=f"gc_{k}", tag="gc")
                    m1 = mm1ps.tile([P, 2, D2], f32, space="PSUM", tag="m1",
                                    name=f"m1_{k}")
                    for jt in range(2):
                        for kt in range(2):
                            nc.tensor.matmul(
                                m1[:, jt, :], lhsT=w1s[kt][jt][:],
                                rhs=hTc[:, kt, :],
                                start=(kt == 0), stop=(kt == 1),
                            )
                        nc.scalar.activation(
                            gsb[:, jt, :], m1[:, jt, :],
                            mybir.ActivationFunctionType.Gelu,
                            bias=b1e_sb[:, jt : jt + 1], scale=1.0,
                        )

                    # y_psum = g^T @ W2 + I @ x + ones x b2  (node-major out)
                    y_sb = wp.tile([P, D2], bf16, tag="y", name=f"y{k}")
                    m2 = mm2ps.tile([P, D2], f32, space="PSUM", tag="m2",
                                    name=f"m2_{k}")
                    for b in range(2):
                        for kt in range(2):
                            nc.tensor.matmul(
                                m2[:, b * D : (b + 1) * D],
                                lhsT=gsb[:, kt, b * D : (b + 1) * D],
                                rhs=w2t[kt][:],
                                start=(kt == 0 and b == 0), stop=False,
                            )
                    nc.tensor.matmul(
                        m2[:], lhsT=ident[:], rhs=xs_sb[:],
                        start=False, stop=False,
                    )
                    nc.tensor.matmul(
                        m2[:], lhsT=ones1[:], rhs=b2r_sb[:],
                        start=False, stop=True,
                    )
                    nc.scalar.copy(y_sb[:], m2[:])
                    nc.sync.dma_start(d_y[k * P : (k + 1) * P, :], y_sb[:])

            pend = None
            for ginfo in groups:
                st = phase_a(ginfo)
                if pend is not None:
                    phase_b(*pend)
                pend = (ginfo, st)
            phase_b(*pend)
    nc.compile()
    return nc


def kernel(x, edge_src, edge_dst, degree, sn_g, sn_b, nn_g, nn_b, W1, b1, W2, b2):
    from concourse.bass_utils import run_bass_kernel_spmd

    x = np.asarray(x)
    Bb, N, D = x.shape
    assert Bb == 2 and D == P, (Bb, N, D)

    struct, percore, shared, tids, N = _prep(x, edge_src, edge_dst)

    key = (struct["NTC"], struct["T0"], struct["T1"],
           struct["NA"], struct["NB"])
    if key not in _CACHE:
        _CACHE.clear()
        _CACHE[key] = _build(struct)
    nc = _CACHE[key]

    W1f = np.asarray(W1, dtype=np.float32)
    b1f = np.asarray(b1, dtype=np.float32).ravel()
    W2f = np.asarray(W2, dtype=ml_dtypes.bfloat16)
    b2f = np.asarray(b2, dtype=np.float32).ravel()
    b2d = np.concatenate([b2f, b2f]).astype(ml_dtypes.bfloat16)
    shared_map = dict(
        xpa=shared["xpa"],
        w1=np.ascontiguousarray(W1f),
        w2=np.ascontiguousarray(W2f),
        b1r=np.ascontiguousarray(b1f.reshape(2, P).T),
        b2r=np.ascontiguousarray(b2d.reshape(1, 2 * P)),
        gx=np.asarray(sn_g, np.float32).reshape(P, 1),
        gn=np.asarray(nn_g, np.float32).reshape(P, 1),
        bx=np.asarray(sn_b, np.float32).reshape(P, 1),
        bn=np.asarray(nn_b, np.float32).reshape(P, 1),
    )
    if shared["xpb"] is not None:
        shared_map["xpb"] = shared["xpb"]

    in_maps = []
    for c in range(NCORES):
        m = dict(shared_map)
        m["xs"] = np.ascontiguousarray(percore["xs"][c])
        m["idx"] = np.ascontiguousarray(percore["idx"][c])
        m["dlb"] = np.ascontiguousarray(percore["dlb"][c])
        in_maps.append(m)

    # the axon-tunneled device occasionally reports
    # NRT_EXEC_UNIT_UNRECOVERABLE on the first attempt; a retry recovers it
    last_exc = None
    for _attempt in range(3):
        try:
            res = run_bass_kernel_spmd(nc, in_maps, core_ids=list(range(NCORES)))
            break
        except Exception as e:  # noqa: BLE001
            last_exc = e
    else:
        raise last_exc

    y = np.empty((Bb, N, D), dtype=np.float32)
    NTC = struct["NTC"]
    for c in range(NCORES):
        yc = np.asarray(res.results[c]["y"], dtype=np.float32)
        for k in range(NTC):
            t = tids[c][k]
            n0 = t * P
            n1 = min(n0 + P, N)
            if n1 <= n0:
                continue
            y[0, n0:n1, :] = yc[k * P : k * P + (n1 - n0), :D]
            y[1, n0:n1, :] = yc[k * P : k * P + (n1 - n0), D:]
    return y


# revision 17
# speedup vs baseline: 1.1021x; 1.1021x over previous
"""Trainium2 Bass kernel for nn_MeshGraphBlock (GNN message-passing block).

Computes, for x:[B,N,D], edges (src,dst):[E], degree:[N]:
    neighbor = scatter_add(x[:, src, :] -> dst) / clip(degree, 1)
    h  = concat(LN(x; sn_g, sn_b), LN(neighbor; nn_g, nn_b))   # [B,N,2D]
    h  = gelu_erf(h @ W1 + b1)                                  # [B,N,2D]
    y  = x + h @ W2 + b2                                        # [B,N,D]

LayerNorm is scale-invariant and `neighbor` only feeds the LN, so the
1/deg factor cancels exactly (up to an eps-term ~1e-4 rel) and is dropped.

Strategy (8 NeuronCores, SPMD):
 - Destination-node tiles (128 nodes each) are assigned to cores via
   sorted round-robin so every core sees the same per-position edge-tile
   counts (the single compiled program is uniform; only data differs).
 - Host pre-sorts edges by dst, packs x (both batches side by side) as a
   bf16 [N,2D] table, and emits per-core gather indices (int16, split in
   two tables to stay under the 32767 index limit).
 - Edge messages are gathered with few, large dma_gather calls (grouped
   across GROUP dst positions; SWDGE ring enlarged via
   dynamic_dma_scratch_size) and scatter-added into 128-dst PSUM
   accumulators via one-hot "selection matrix" matmuls on the PE.
 - All matmul operands are bf16 (1 PE cycle/row); residual x and the b2
   bias are folded into the mm2 PSUM chain as extra matmuls; rstd is
   computed on DVE with a fused (var+eps)^-0.5 tensor_scalar (no
   activation-table thrash).
"""

import math

import numpy as np
import ml_dtypes

P = 128
NCORES = 8
SPLIT = 32768           # int16 gather-index limit
MAX_TILES_PER_CALL = 24  # 3072 idxs per dma_gather (enlarged SWDGE ring)
TUNE = dict(gpool=3, spool=6, wp=6, hp=4, group=3, scratch=65536)

_CACHE = {}


def _prep(x, edge_src, edge_dst):
    """Host-side sharding. Returns (structure, per-core inputs, assembly map)."""
    Bb, N, D = x.shape
    es = np.asarray(edge_src).astype(np.int64).ravel()
    ed = np.asarray(edge_dst).astype(np.int64).ravel()

    ntiles = math.ceil(N / P)
    ntiles_pad = math.ceil(ntiles / NCORES) * NCORES
    NTC = ntiles_pad // NCORES
    GROUP = TUNE["group"]

    order = np.argsort(ed, kind="stable")
    ed_s = ed[order]
    es_s = es[order]
    bounds = np.searchsorted(ed_s, np.arange(ntiles_pad + 1) * P)

    counts = bounds[1:] - bounds[:-1]
    ranked = np.argsort(-counts, kind="stable")
    # tile ranked[i] -> core i % 8, position i // 8
    tids = [[0] * NTC for _ in range(NCORES)]
    for i, t in enumerate(ranked):
        tids[i % NCORES][i // NCORES] = int(t)

    # per (core, pos): split into G0 (src < SPLIT) and G1
    g0i, g1i, dli = {}, {}, {}
    for c in range(NCORES):
        for k in range(NTC):
            t = tids[c][k]
            a, b = bounds[t], bounds[t + 1]
            srcs = es_s[a:b]
            dloc = (ed_s[a:b] - t * P).astype(np.int64)
            m0 = srcs < SPLIT
            g0i[c, k] = srcs[m0].astype(np.int64)
            g1i[c, k] = (srcs[~m0] - SPLIT).astype(np.int64)
            dli[c, k] = (dloc[m0], dloc[~m0])

    T0 = [max(math.ceil(len(g0i[c, k]) / P) for c in range(NCORES)) for k in range(NTC)]
    T1 = [max(math.ceil(len(g1i[c, k]) / P) for c in range(NCORES)) for k in range(NTC)]

    # Group-level layout: for each group of GROUP positions, the tile
    # stream is [k0 A-tiles, k1 A-tiles, ..., k0 B-tiles, k1 B-tiles, ...]
    # so one (or few) gather calls per source table cover the whole group.
    TTOT = sum(T0) + sum(T1)
    idx_flat = np.zeros((NCORES, TTOT * P), dtype=np.int16)
    dl_flat = np.full((NCORES, TTOT * P), -1.0, dtype=np.float32)
    groups = []   # per group: dict(calls=[(src, slot_off, ntiles, idx_off)],
                  #                pos=[(k, a_off, b_off)], tg=total tiles)
    tile_off = 0  # global tile counter (indexes dlb columns / idx stream)
    for k0 in range(0, NTC, GROUP):
        gs = min(GROUP, NTC - k0)
        ginfo = dict(calls=[], pos=[], base=tile_off)
        # A tiles then B tiles, each position-ordered
        offs = {}
        so = 0
        for grp, Tarr in ((0, T0), (1, T1)):
            grp_start_tile = tile_off
            grp_start_slot = so
            ntile_grp = 0
            for gi in range(gs):
                k = k0 + gi
                T = Tarr[k]
                offs[k, grp] = so
                for c in range(NCORES):
                    ii = g0i[c, k] if grp == 0 else g1i[c, k]
                    dd = dli[c, k][grp]
                    o = tile_off * P
                    idx_flat[c, o : o + len(ii)] = ii.astype(np.int16)
                    dl_flat[c, o : o + len(dd)] = dd.astype(np.float32)
                tile_off += T
                so += T
                ntile_grp += T
            # per-position calls (fine granularity for pipelining); the
            # enlarged SWDGE ring lets each position be a single call
            pos_off = 0
            for gi in range(gs):
                T = Tarr[k0 + gi]
                done = 0
                while done < T:
                    nt = min(T - done, MAX_TILES_PER_CALL)
                    ginfo["calls"].append(
                        ("A" if grp == 0 else "B",
                         grp_start_slot + pos_off + done,
                         nt, (grp_start_tile + pos_off + done) * P)
                    )
                    done += nt
                pos_off += T
        for gi in range(gs):
            k = k0 + gi
            ginfo["pos"].append((k, offs[k, 0], offs[k, 1]))
        ginfo["tg"] = so
        groups.append(ginfo)
    assert tile_off == TTOT

    # wrapped int16 idx layout: [128, TTOT*P/16]
    idx_wrapped = np.stack(
        [np.tile(idx_flat[c].reshape(-1, 16).T, (8, 1)) for c in range(NCORES)]
    )
    dlb = np.stack(
        [np.ascontiguousarray(dl_flat[c].reshape(TTOT, P).T)
         for c in range(NCORES)]
    )  # [NCORES, 128, TTOT] float32

    # per-core x slices ([NTC*128, 2D]) bf16
    xs = np.zeros((NCORES, NTC * P, 2 * D), dtype=ml_dtypes.bfloat16)
    xf = np.asarray(x, dtype=np.float32)
    for c in range(NCORES):
        for k in range(NTC):
            t = tids[c][k]
            n0 = t * P
            n1 = min(n0 + P, N)
            if n1 <= n0:
                continue
            xs[c, k * P : k * P + (n1 - n0), :D] = xf[0, n0:n1, :]
            xs[c, k * P : k * P + (n1 - n0), D:] = xf[1, n0:n1, :]

    # packed gather tables (both batches side by side), bf16
    xpack = np.concatenate([xf[0], xf[1]], axis=1).astype(ml_dtypes.bfloat16)
    xpa = np.ascontiguousarray(xpack[:SPLIT])
    xpb = np.ascontiguousarray(xpack[SPLIT:]) if N > SPLIT else None

    slots_max = max(t0 + t1 for t0, t1 in zip(T0, T1))
    tg_max = max(g["tg"] for g in groups)
    struct = dict(NTC=NTC, T0=tuple(T0), T1=tuple(T1), TTOT=TTOT,
                  groups=groups, slots_max=slots_max, tg_max=tg_max,
                  NA=xpa.shape[0], NB=(xpb.shape[0] if xpb is not None else 0),
                  D=D, Bb=Bb)
    percore = dict(idx=idx_wrapped, dlb=dlb, xs=xs)
    shared = dict(xpa=xpa, xpb=xpb)
    return struct, percore, shared, tids, N


def _build(struct):
    import concourse.bacc as bacc
    import concourse.tile as tile
    from concourse import bass, mybir
    from concourse.masks import make_identity

    NTC, T0, T1, TTOT = struct["NTC"], struct["T0"], struct["T1"], struct["TTOT"]
    groups = struct["groups"]
    D = struct["D"]
    D2 = 2 * D
    TOTCOLS = TTOT * P // 16
    slots_max = struct["slots_max"]
    tg_max = struct["tg_max"]
    f32, bf16, i16 = mybir.dt.float32, mybir.dt.bfloat16, mybir.dt.int16

    nc = bacc.Bacc("TRN2", target_bir_lowering=False, debug=False,
                   dynamic_dma_scratch_size=TUNE["scratch"])
    d_xpa = nc.dram_tensor("xpa", [struct["NA"], D2], bf16, kind="ExternalInput")
    d_xpb = (nc.dram_tensor("xpb", [struct["NB"], D2], bf16, kind="ExternalInput")
             if struct["NB"] else None)
    d_xs = nc.dram_tensor("xs", [NTC * P, D2], bf16, kind="ExternalInput")
    d_idx = nc.dram_tensor("idx", [P, TOTCOLS], i16, kind="ExternalInput")
    d_dlb = nc.dram_tensor("dlb", [P, TTOT], f32, kind="ExternalInput")
    d_w1 = nc.dram_tensor("w1", [D2, D2], f32, kind="ExternalInput")
    d_w2 = nc.dram_tensor("w2", [D2, D], bf16, kind="ExternalInput")
    d_b1 = nc.dram_tensor("b1r", [P, 2], f32, kind="ExternalInput")
    d_b2 = nc.dram_tensor("b2r", [1, D2], bf16, kind="ExternalInput")
    d_gx = nc.dram_tensor("gx", [P, 1], f32, kind="ExternalInput")
    d_gn = nc.dram_tensor("gn", [P, 1], f32, kind="ExternalInput")
    d_bx = nc.dram_tensor("bx", [P, 1], f32, kind="ExternalInput")
    d_bn = nc.dram_tensor("bn", [P, 1], f32, kind="ExternalInput")
    d_y = nc.dram_tensor("y", [NTC * P, D2], bf16, kind="ExternalOutput")

    with tile.TileContext(nc) as tc:
        with (
            tc.tile_pool(name="const", bufs=1) as cp,
            tc.tile_pool(name="gath", bufs=TUNE["gpool"]) as gpool,
            tc.tile_pool(name="sel", bufs=TUNE["spool"]) as spool,
            tc.tile_pool(name="work", bufs=TUNE["wp"]) as wp,
            tc.tile_pool(name="ht", bufs=TUNE["hp"]) as hp,
            tc.tile_pool(name="nbps", bufs=2, space="PSUM") as nbps,
            tc.tile_pool(name="trps", bufs=2, space="PSUM") as trps,
            tc.tile_pool(name="mm1ps", bufs=2, space="PSUM") as mm1ps,
            tc.tile_pool(name="mm2ps", bufs=2, space="PSUM") as mm2ps,
        ):
            # ---- one-time constants ----
            idx_sb = cp.tile([P, TOTCOLS], i16)
            nc.sync.dma_start(idx_sb[:], d_idx.ap())
            dlb_sb = cp.tile([P, TTOT], f32)
            nc.sync.dma_start(dlb_sb[:], d_dlb.ap())

            ident = cp.tile([P, P], bf16)
            make_identity(nc, ident[:])
            iota1 = cp.tile([P, P], bf16)
            nc.gpsimd.iota(iota1[:], pattern=[[1, P]], base=0,
                           channel_multiplier=0,
                           allow_small_or_imprecise_dtypes=True)

            gx_sb = cp.tile([P, 1], f32); nc.sync.dma_start(gx_sb[:], d_gx.ap())
            gn_sb = cp.tile([P, 1], f32); nc.sync.dma_start(gn_sb[:], d_gn.ap())
            bx_sb = cp.tile([P, 1], f32); nc.sync.dma_start(bx_sb[:], d_bx.ap())
            bn_sb = cp.tile([P, 1], f32); nc.sync.dma_start(bn_sb[:], d_bn.ap())
            b1r_sb = cp.tile([P, 2], f32); nc.sync.dma_start(b1r_sb[:], d_b1.ap())
            b2r_sb = cp.tile([1, D2], bf16); nc.sync.dma_start(b2r_sb[:], d_b2.ap())
            ones1 = cp.tile([1, P], bf16)
            nc.vector.memset(ones1[:], 1.0)

            # W1 tiles [k-tile][j-tile], gamma-scaled bf16 copies, W2 bf16
            w1t = [[cp.tile([P, P], f32, name=f"w1t{kt}{jt}") for jt in range(2)]
                   for kt in range(2)]
            w1s = [[cp.tile([P, P], bf16, name=f"w1s{kt}{jt}") for jt in range(2)]
                   for kt in range(2)]
            gam = [gx_sb, gn_sb]
            for kt in range(2):
                for jt in range(2):
                    nc.sync.dma_start(
                        w1t[kt][jt][:],
                        d_w1[kt * P : (kt + 1) * P, jt * P : (jt + 1) * P],
                    )
                    nc.vector.tensor_scalar_mul(
                        w1s[kt][jt][:], w1t[kt][jt][:], gam[kt][:]
                    )
            w2t = [cp.tile([P, P], bf16, name=f"w2t{kt}") for kt in range(2)]
            for kt in range(2):
                nc.sync.dma_start(w2t[kt][:], d_w2[kt * P : (kt + 1) * P, :])

            # b1_eff = b1 + beta_cat @ W1  (per-partition layout [128, j-tile])
            bet = [bx_sb, bn_sb]
            b1b_ps = mm1ps.tile([P, 2], f32, space="PSUM", tag="m1")
            for jt in range(2):
                for kt in range(2):
                    nc.tensor.matmul(
                        b1b_ps[:, jt : jt + 1], lhsT=w1t[kt][jt][:],
                        rhs=bet[kt][:], start=(kt == 0), stop=(kt == 1),
                    )
            b1e_sb = cp.tile([P, 2], f32)
            nc.vector.tensor_add(b1e_sb[:], b1b_ps[:], b1r_sb[:])

            i32 = mybir.dt.int32
            GROUP = max(len(g["pos"]) for g in groups)
            magic = cp.tile([P, 4 * GROUP], i32)
            nc.vector.memset(magic[:], 0x5F3759DF)

            # ---- main loop: software-pipelined emission ----
            # Phase A of group i is emitted BEFORE phase B of group i-1 so
            # every engine queue has the next group's work ahead of any
            # instruction stalled on the current group's rstd barrier.
            def phase_a(ginfo):
                base = ginfo["base"]
                gs = len(ginfo["pos"])
                nb_t, xs_t = {}, {}
                mvg = wp.tile([P, 4 * gs, 2], f32, tag="mvg", bufs=2,
                              name=f"mvg{base}")

                # group gather: few large calls
                g = gpool.tile([P, tg_max, D2], bf16, tag="g", name=f"g{base}")
                for (srcg, so, nt, io) in ginfo["calls"]:
                    src_t = d_xpa if srcg == "A" else d_xpb
                    nc.gpsimd.dma_gather(
                        g[:, so : so + nt, :], src_t.ap(),
                        idx_sb[:, io // 16 : (io + nt * P) // 16],
                        nt * P, nt * P, D2, single_packet=False,
                    )

                # phase A: aggregate neighbors + stats
                for gi, (k, a_off, b_off) in enumerate(ginfo["pos"]):
                    slots = T0[k] + T1[k]
                    nb_sb = wp.tile([P, D2], bf16, tag="nb", bufs=2 * TUNE["group"] + 1,
                                    name=f"nb{k}")
                    nb_t[k] = nb_sb
                    if slots == 0:
                        nc.vector.memset(nb_sb[:], 0.0)
                    else:
                        # (g-slot index, dlb column) per slot of this position
                        slist = (
                            [(a_off + t, base + a_off + t) for t in range(T0[k])]
                            + [(b_off + t, base + b_off + t) for t in range(T1[k])]
                        )
                        S = spool.tile([P, slots_max, P], bf16, tag="S",
                                       name=f"S{k}")
                        for i, (gslot, dcol) in enumerate(slist):
                            nc.vector.tensor_scalar(
                                out=S[:, i, :],
                                in0=iota1[:],
                                scalar1=dlb_sb[:, dcol : dcol + 1],
                                scalar2=None,
                                op0=mybir.AluOpType.is_equal,
                            )
                        nb_ps = nbps.tile([P, D2], f32, space="PSUM", tag="nbp",
                                          name=f"nbp{k}")
                        for i, (gslot, dcol) in enumerate(slist):
                            nc.tensor.matmul(
                                nb_ps[:], lhsT=S[:, i, :], rhs=g[:, gslot, :],
                                start=(i == 0), stop=(i == len(slist) - 1),
                            )
                        nc.scalar.copy(nb_sb[:], nb_ps[:])

                    xs_sb = wp.tile([P, D2], bf16, tag="xs", bufs=2 * TUNE["group"] + 1,
                                    name=f"xs{k}")
                    xs_t[k] = xs_sb
                    nc.sync.dma_start(xs_sb[:], d_xs[k * P : (k + 1) * P, :])
                    stx = wp.tile([P, 2, 6], f32, tag="stx", name=f"stx{k}")
                    stn = wp.tile([P, 2, 6], f32, tag="stn", name=f"stn{k}")
                    for b in range(2):
                        nc.vector.bn_stats(stx[:, b, :], xs_sb[:, b * D : (b + 1) * D])
                        nc.vector.bn_stats(stn[:, b, :], nb_sb[:, b * D : (b + 1) * D])
                    nc.vector.bn_aggr(mvg[:, 4 * gi + 0, :], stx[:, 0:1, :])
                    nc.vector.bn_aggr(mvg[:, 4 * gi + 1, :], stx[:, 1:2, :])
                    nc.vector.bn_aggr(mvg[:, 4 * gi + 2, :], stn[:, 0:1, :])
                    nc.vector.bn_aggr(mvg[:, 4 * gi + 3, :], stn[:, 1:2, :])

                # group-level rstd = (var + eps)^-0.5 on DVE via the
                # bit-trick rsqrt + 2 Newton iterations (no act-table load)
                rsg = wp.tile([P, 4 * gs], f32, tag="rsg", bufs=2, name=f"rsg{base}")
                veps = wp.tile([P, 4 * gs], f32, tag="veps", bufs=2,
                               name=f"veps{base}")
                ytmp = wp.tile([P, 4 * gs], f32, tag="ytmp", bufs=2,
                               name=f"ytmp{base}")
                ttmp = wp.tile([P, 4 * gs], f32, tag="ttmp", bufs=2,
                               name=f"ttmp{base}")
                nc.vector.tensor_scalar_add(veps[:], mvg[:, :, 1], 1e-5)
                # y0 = bitcast(magic - (bitcast(veps) >> 1))
                nc.vector.tensor_scalar(
                    out=ytmp[:].bitcast(i32), in0=veps[:].bitcast(i32),
                    scalar1=1, scalar2=None,
                    op0=mybir.AluOpType.arith_shift_right,
                )
                nc.vector.tensor_tensor(
                    out=ytmp[:].bitcast(i32), in0=magic[:, : 4 * gs],
                    in1=ytmp[:].bitcast(i32), op=mybir.AluOpType.subtract,
                )
                for _ in range(1):  # y <- y * (1.5 - 0.5 * veps * y^2)
                    nc.vector.tensor_tensor(
                        out=ttmp[:], in0=ytmp[:], in1=ytmp[:],
                        op=mybir.AluOpType.mult,
                    )
                    nc.vector.tensor_tensor(
                        out=ttmp[:], in0=ttmp[:], in1=veps[:],
                        op=mybir.AluOpType.mult,
                    )
                    nc.vector.tensor_scalar(
                        out=ttmp[:], in0=ttmp[:], scalar1=-0.5, scalar2=1.5,
                        op0=mybir.AluOpType.mult, op1=mybir.AluOpType.add,
                    )
                    nc.vector.tensor_tensor(
                        out=ytmp[:], in0=ytmp[:], in1=ttmp[:],
                        op=mybir.AluOpType.mult,
                    )
                nc.vector.tensor_copy(rsg[:], ytmp[:])
                return dict(nb_t=nb_t, xs_t=xs_t, mvg=mvg, rsg=rsg)

            # phase B: normalize + MLP + residual
            def phase_b(ginfo, st):
                nb_t, xs_t = st["nb_t"], st["xs_t"]
                mvg, rsg = st["mvg"], st["rsg"]
                for gi, (k, a_off, b_off) in enumerate(ginfo["pos"]):
                    nb_sb, xs_sb = nb_t[k], xs_t[k]
                    hx = wp.tile([P, D2], bf16, tag="hx", name=f"hx{k}")
                    hn = wp.tile([P, D2], bf16, tag="hn", name=f"hn{k}")
                    for b in range(2):
                        nc.vector.tensor_scalar(
                            out=hx[:, b * D : (b + 1) * D],
                            in0=xs_sb[:, b * D : (b + 1) * D],
                            scalar1=mvg[:, 4 * gi + b, 0:1],
                            scalar2=rsg[:, 4 * gi + b : 4 * gi + b + 1],
                            op0=mybir.AluOpType.subtract,
                            op1=mybir.AluOpType.mult,
                        )
                        nc.vector.tensor_scalar(
                            out=hn[:, b * D : (b + 1) * D],
                            in0=nb_sb[:, b * D : (b + 1) * D],
                            scalar1=mvg[:, 4 * gi + 2 + b, 0:1],
                            scalar2=rsg[:, 4 * gi + 2 + b : 4 * gi + 3 + b],
                            op0=mybir.AluOpType.subtract,
                            op1=mybir.AluOpType.mult,
                        )

                    # feature-major h via PE transposes into ONE [128,512]
                    # PSUM tile (1 bank -> true depth-2 pipelining); halves
                    # kt=0 -> x, kt=1 -> nb, both batches side by side
                    hTc = hp.tile([P, 2, D2], bf16, name=f"hTc_{k}", tag="hTc")
                    tp = trps.tile([P, 2, D2], bf16, space="PSUM", tag="tr",
                                   name=f"tr_{k}")
                    for kt, srct in ((0, hx), (1, hn)):
                        for b in range(2):
                            nc.tensor.transpose(
                                tp[:, kt, b * D : (b + 1) * D],
                                srct[:, b * D : (b + 1) * D], ident[:],
                            )
                    nc.scalar.copy(hTc[:], tp[:])

                    gsb = hp.tile([P, 2, D2], bf16, name=f"gc_{k}", tag="gc")
                    m1 = mm1ps.tile([P, 2, D2], f32, space="PSUM", tag="m1",
                                    name=f"m1_{k}")
                    for jt in range(2):
                        for kt in range(2):
                            nc.tensor.matmul(
                                m1[:, jt, :], lhsT=w1s[kt][jt][:],
                                rhs=hTc[:, kt, :],
                                start=(kt == 0), stop=(kt == 1),
                            )
                        nc.scalar.activation(
                            gsb[:, jt, :], m1[:, jt, :],
                            mybir.ActivationFunctionType.Gelu,
                            bias=b1e_sb[:, jt : jt + 1], scale=1.0,
                        )

                    # y_psum = g^T @ W2 + I @ x + ones x b2  (node-major out)
                    y_sb = wp.tile([P, D2], bf16, tag="y", name=f"y{k}")
                    m2 = mm2ps.tile([P, D2], f32, space="PSUM", tag="m2",
                                    name=f"m2_{k}")
                    for b in range(2):
                        for kt in range(2):
                            nc.tensor.matmul(
                                m2[:, b * D : (b + 1) * D],
                                lhsT=gsb[:, kt, b * D : (b + 1) * D],
                                rhs=w2t[kt][:],
                                start=(kt == 0 and b == 0), stop=False,
                            )
                    nc.tensor.matmul(
                        m2[:], lhsT=ident[:], rhs=xs_sb[:],
                        start=False, stop=False,
                    )
                    nc.tensor.matmul(
                        m2[:], lhsT=ones1[:], rhs=b2r_sb[:],
                        start=False, stop=True,
                    )
                    nc.scalar.copy(y_sb[:], m2[:])
                    nc.sync.dma_start(d_y[k * P : (k + 1) * P, :], y_sb[:])

            for ginfo in groups:
                st = phase_a(ginfo)
                phase_b(ginfo, st)
    nc.compile()
    return nc


def kernel(x, edge_src, edge_dst, degree, sn_g, sn_b, nn_g, nn_b, W1, b1, W2, b2):
    from concourse.bass_utils import run_bass_kernel_spmd

    x = np.asarray(x)
    Bb, N, D = x.shape
    assert Bb == 2 and D == P, (Bb, N, D)

    struct, percore, shared, tids, N = _prep(x, edge_src, edge_dst)

    key = (struct["NTC"], struct["T0"], struct["T1"],
           struct["NA"], struct["NB"])
    if key not in _CACHE:
        _CACHE.clear()
        _CACHE[key] = _build(struct)
    nc = _CACHE[key]

    W1f = np.asarray(W1, dtype=np.float32)
    b1f = np.asarray(b1, dtype=np.float32).ravel()
    W2f = np.asarray(W2, dtype=ml_dtypes.bfloat16)
    b2f = np.asarray(b2, dtype=np.float32).ravel()
    b2d = np.concatenate([b2f, b2f]).astype(ml_dtypes.bfloat16)
    shared_map = dict(
        xpa=shared["xpa"],
        w1=np.ascontiguousarray(W1f),
        w2=np.ascontiguousarray(W2f),
        b1r=np.ascontiguousarray(b1f.reshape(2, P).T),
        b2r=np.ascontiguousarray(b2d.reshape(1, 2 * P)),
        gx=np.asarray(sn_g, np.float32).reshape(P, 1),
        gn=np.asarray(nn_g, np.float32).reshape(P, 1),
        bx=np.asarray(sn_b, np.float32).reshape(P, 1),
        bn=np.asarray(nn_b, np.float32).reshape(P, 1),
    )
    if shared["xpb"] is not None:
        shared_map["xpb"] = shared["xpb"]

    in_maps = []
    for c in range(NCORES):
        m = dict(shared_map)
        m["xs"] = np.ascontiguousarray(percore["xs"][c])
        m["idx"] = np.ascontiguousarray(percore["idx"][c])
        m["dlb"] = np.ascontiguousarray(percore["dlb"][c])
        in_maps.append(m)

    # the axon-tunneled device occasionally reports
    # NRT_EXEC_UNIT_UNRECOVERABLE on the first attempt; a retry recovers it
    last_exc = None
    for _attempt in range(3):
        try:
            res = run_bass_kernel_spmd(nc, in_maps, core_ids=list(range(NCORES)))
            break
        except Exception as e:  # noqa: BLE001
            last_exc = e
    else:
        raise last_exc

    y = np.empty((Bb, N, D), dtype=np.float32)
    NTC = struct["NTC"]
    for c in range(NCORES):
        yc = np.asarray(res.results[c]["y"], dtype=np.float32)
        for k in range(NTC):
            t = tids[c][k]
            n0 = t * P
            n1 = min(n0 + P, N)
            if n1 <= n0:
                continue
            y[0, n0:n1, :] = yc[k * P : k * P + (n1 - n0), :D]
            y[1, n0:n1, :] = yc[k * P : k * P + (n1 - n0), D:]
    return y
